# revision 1
# baseline (speedup 1.0000x reference)
"""Trainium2 Bass kernel for DGNRNetwork (2-layer TransformerConv GNN + MLPs).

Strategy (8 NeuronCores, graph/data parallel):
  - Nodes padded to N_PAD=50176 and sharded by contiguous range: core c owns
    nodes [c*6272, (c+1)*6272), i.e. 49 blocks of 128 dst nodes per core.
  - Edges are partitioned by dst shard on host, sorted by (dst block, src
    half, src), padded so every (block, src-half) group is a whole number of
    128-edge tiles (tile counts uniform across cores -> one SPMD program).
  - Per edge-block: k||v rows and q rows are fetched with indirect DMA
    (dma_gather); per-edge attention logits are computed on the Vector
    engine; exp on Scalar engine; the segment-softmax denominator and the
    weighted sum over incoming edges are ONE accumulated TensorE matmul with
    a host-precomputed one-hot scatter matrix S (S[e, d] = edge e's dst is
    block-node d). Padding edges have all-zero S rows so they drop out.
  - Small weights are replicated; k/v row tables are exchanged with an
    AllGather between the conv layers; the tiny Q-head is computed
    redundantly and combined with a masked AllReduce.
"""

import sys

sys.path.insert(0, "/opt/trn_rl_repo")

import numpy as np
import ml_dtypes

import concourse.bacc as bacc
import concourse.bass as bass
import concourse.mybir as mybir
import concourse.tile as tile
from concourse import bass_utils, library_config

F32 = mybir.dt.float32
BF16 = mybir.dt.bfloat16
I16 = mybir.dt.int16

N_CORES = 8


class Cfg:
    def __init__(self, n_nodes=50000, nblk=49, b=64, edge_bf16=True):
        self.N = n_nodes
        self.NBLK = nblk                 # dst blocks per core
        self.SHARD = nblk * 128          # nodes per core
        self.N_PAD = 8 * self.SHARD
        self.HALF = self.N_PAD // 2      # kv table split (int16 gather idx)
        self.B = b                       # batch (selected nodes)
        self.F_IN = 128
        self.H = 32
        self.HD = 128
        self.HEADS = 4
        self.EPS = 1e-16
        self.SCALE = 1.0 / np.sqrt(32.0)
        self.edge_bf16 = edge_bf16
        self.EDT = BF16 if edge_bf16 else F32
        self.EDT_NP = ml_dtypes.bfloat16 if edge_bf16 else np.float32
        assert self.N <= self.N_PAD and self.HALF < 32768


# --------------------------------------------------------------------------
# host-side preprocessing
# --------------------------------------------------------------------------


def _wrap16(values, slots):
    """dma_gather idx layout: idx i lives at [i % 16, i // 16], replicated
    across the eight 16-partition groups."""
    arr = np.zeros((16, slots // 16), dtype=np.int16)
    arr[np.arange(len(values)) % 16, np.arange(len(values)) // 16] = values
    return np.tile(arr, (8, 1))


def _prep_edges(cfg, edge_index):
    src = np.ascontiguousarray(edge_index[0]).astype(np.int64)
    dst = np.ascontiguousarray(edge_index[1]).astype(np.int64)
    core = dst // cfg.SHARD
    blk = (dst % cfg.SHARD) // 128
    hi = (src >= cfg.HALF).astype(np.int64)

    cnt = np.zeros((N_CORES, cfg.NBLK, 2), np.int64)
    np.add.at(cnt, (core, blk, hi), 1)
    t_lo = np.maximum(1, (cnt[:, :, 0].max(0) + 127) // 128)  # [NBLK]
    t_hi = np.maximum(1, (cnt[:, :, 1].max(0) + 127) // 128)
    t_all = t_lo + t_hi
    blk_off = np.zeros(cfg.NBLK + 1, np.int64)
    blk_off[1:] = np.cumsum(t_all * 128)
    slots = int(blk_off[-1])
    assert slots % 16 == 0

    order = np.lexsort((src, hi, blk, core))
    s_src, s_dst, s_core, s_blk, s_hi = (
        src[order], dst[order], core[order], blk[order], hi[order])

    per_core = []
    for c in range(N_CORES):
        m = s_core == c
        csrc, cdst, cblk, chi = s_src[m], s_dst[m], s_blk[m], s_hi[m]
        grp = cblk * 2 + chi  # non-decreasing (sorted)
        gcounts = np.bincount(grp, minlength=2 * cfg.NBLK)
        gstarts = np.zeros(2 * cfg.NBLK, np.int64)
        gstarts[1:] = np.cumsum(gcounts)[:-1]
        rank = np.arange(len(grp)) - gstarts[grp]
        slot = blk_off[cblk] + chi * (t_lo[cblk] * 128) + rank

        kv_val = np.where(chi == 1, csrc - cfg.HALF, csrc)
        kv_idx = np.zeros(slots, np.int64)
        kv_idx[slot] = kv_val
        qi_idx = np.zeros(slots, np.int64)
        qi_idx[slot] = cdst - c * cfg.SHARD

        S = np.zeros((128, slots), cfg.EDT_NP)
        scol = (slot // 128) * 128 + (cdst % 128)
        S[slot % 128, scol] = 1.0

        per_core.append(dict(kv_idx=_wrap16(kv_idx, slots),
                             qi_idx=_wrap16(qi_idx, slots), S=S))
    return per_core, t_lo.tolist(), t_hi.tolist(), blk_off.tolist(), slots


def _prep_inputs(cfg, inputs):
    x = np.asarray(inputs["x"], np.float32)
    idx = np.asarray(inputs["idx"]).astype(np.int64)
    f32 = lambda k: np.ascontiguousarray(np.asarray(inputs[k], np.float32))

    xp = np.zeros((cfg.N_PAD, cfg.F_IN), np.float32)
    xp[: cfg.N] = x

    per_core_e, t_lo, t_hi, blk_off, slots = _prep_edges(cfg, inputs["edge_index"])

    wkv1 = np.ascontiguousarray(
        np.concatenate([f32("c1_wk"), f32("c1_wv")], axis=1))     # [32,256]
    bkv1 = np.ascontiguousarray(
        np.concatenate([f32("c1_bk"), f32("c1_bv")])[None, :])    # [1,256]
    wkv2 = np.ascontiguousarray(
        np.concatenate([f32("c2_wk"), f32("c2_wv")], axis=1))     # [128,256]
    bkv2 = np.ascontiguousarray(
        np.concatenate([f32("c2_bk"), f32("c2_bv")])[None, :])
    qw1 = f32("q_w1")                                              # [288,128]
    bpad = ((cfg.B + 127) // 128) * 128

    in_maps = []
    for c in range(N_CORES):
        shard = slice(c * cfg.SHARD, (c + 1) * cfg.SHARD)
        own = (idx // cfg.SHARD) == c
        idx_loc = np.where(own, idx - c * cfg.SHARD, 0)
        im = dict(
            xT=np.ascontiguousarray(xp[shard].T),          # [128, SHARD]
            enc_w1=f32("enc_w1"),
            enc_b1=f32("enc_b1").reshape(32, 1),
            enc_w2=f32("enc_w2"),
            enc_b2c=f32("enc_b2").reshape(32, 1),
            enc_b2r=f32("enc_b2").reshape(1, 32),
            wq1=f32("c1_wq"), bq1=np.ascontiguousarray(f32("c1_bq")[None, :]),
            wkv1=wkv1, bkv1=bkv1,
            wq2=f32("c2_wq"), bq2=np.ascontiguousarray(f32("c2_bq")[None, :]),
            wkv2=wkv2, bkv2=bkv2,
            qw1a=np.ascontiguousarray(qw1[0:32]),
            qw1b=np.ascontiguousarray(qw1[32:160]),
            qw1c=np.ascontiguousarray(qw1[160:288]),
            qb1=f32("q_b1").reshape(128, 1),
            qw2=f32("q_w2"),
            qb2=f32("q_b2").reshape(1, 2),
            ones128=np.ones((1, 128), np.float32),
            id128=np.eye(128, dtype=np.float32),
            kv_idx=per_core_e[c]["kv_idx"],
            qi_idx=per_core_e[c]["qi_idx"],
            S_all=per_core_e[c]["S"],
            idx_x=_wrap16(idx_loc, bpad),
            own_mask=own.astype(np.float32).reshape(cfg.B, 1),
        )
        in_maps.append(im)
    return in_maps, t_lo, t_hi, blk_off, slots


# --------------------------------------------------------------------------
# device program
# --------------------------------------------------------------------------


def build_program(cfg, t_lo, t_hi, blk_off, slots):
    nc = bacc.Bacc("TRN2", target_bir_lowering=False, debug=False,
                   num_devices=N_CORES)
    EDT = cfg.EDT
    NB, SH = cfg.NBLK, cfg.SHARD
    RG = [list(range(N_CORES))]
    RELU = mybir.ActivationFunctionType.Relu
    COPY = mybir.ActivationFunctionType.Copy
    EXP = mybir.ActivationFunctionType.Exp

    def din(name, shape, dt=F32):
        return nc.dram_tensor(name, list(shape), dt, kind="ExternalInput").ap()

    xT = din("xT", [128, SH])
    enc_w1 = din("enc_w1", [128, 32]); enc_b1 = din("enc_b1", [32, 1])
    enc_w2 = din("enc_w2", [32, 32]); enc_b2c = din("enc_b2c", [32, 1])
    enc_b2r = din("enc_b2r", [1, 32])
    wq1 = din("wq1", [32, 128]); bq1 = din("bq1", [1, 128])
    wkv1 = din("wkv1", [32, 256]); bkv1 = din("bkv1", [1, 256])
    wq2 = din("wq2", [128, 128]); bq2 = din("bq2", [1, 128])
    wkv2 = din("wkv2", [128, 256]); bkv2 = din("bkv2", [1, 256])
    qw1a = din("qw1a", [32, 128]); qw1b = din("qw1b", [128, 128])
    qw1c = din("qw1c", [128, 128]); qb1 = din("qb1", [128, 1])
    qw2 = din("qw2", [128, 2]); qb2 = din("qb2", [1, 2])
    ones128 = din("ones128", [1, 128]); id128 = din("id128", [128, 128])
    kv_idx_d = din("kv_idx", [128, slots // 16], I16)
    qi_idx_d = din("qi_idx", [128, slots // 16], I16)
    S_d = din("S_all", [128, slots], EDT)
    bpad = ((cfg.B + 127) // 128) * 128
    idx_x_d = din("idx_x", [128, bpad // 16], I16)
    own_mask_d = din("own_mask", [cfg.B, 1])
    out_d = nc.dram_tensor("out", [cfg.B, 2], F32, kind="ExternalOutput").ap()

    with tile.TileContext(nc) as tc:
        with (
            tc.tile_pool(name="const", bufs=1) as cpool,
            tc.tile_pool(name="work", bufs=2) as wpool,
            tc.tile_pool(name="work1", bufs=1) as w1pool,
            tc.tile_pool(name="small", bufs=3) as spool,
            tc.tile_pool(name="psA", bufs=2, space="PSUM") as psA,
            tc.tile_pool(name="psB", bufs=2, space="PSUM") as psB,
            tc.tile_pool(name="psC", bufs=2, space="PSUM") as psC,
            tc.tile_pool(name="dram", bufs=1, space="DRAM") as dpool,
        ):
            nc.gpsimd.load_library(library_config.mlp)

            def ld(ap, shape, dt=F32, nm=None):
                t = cpool.tile(shape, dt, name=nm or ("ld_" + ap.tensor.name))
                nc.sync.dma_start(t[:], ap[:])
                return t

            w_enc1 = ld(enc_w1, [128, 32]); b_enc1 = ld(enc_b1, [32, 1])
            w_enc2 = ld(enc_w2, [32, 32]); b_enc2c = ld(enc_b2c, [32, 1])
            b_enc2r = ld(enc_b2r, [1, 32])
            w_q1 = ld(wq1, [32, 128]); b_q1 = ld(bq1, [1, 128])
            w_kv1 = ld(wkv1, [32, 256]); b_kv1 = ld(bkv1, [1, 256])
            w_q2 = ld(wq2, [128, 128]); b_q2 = ld(bq2, [1, 128])
            w_kv2 = ld(wkv2, [128, 256]); b_kv2 = ld(bkv2, [1, 256])
            w_qha = ld(qw1a, [32, 128]); w_qhb = ld(qw1b, [128, 128])
            w_qhc = ld(qw1c, [128, 128])
            b_qh = ld(qb1, [128, 1]); w_qh2 = ld(qw2, [128, 2])
            b_qh2 = ld(qb2, [1, 2])
            ones_s = ld(ones128, [1, 128]); id_s = ld(id128, [128, 128])
            nidx16 = (cfg.B + 15) // 16
            idxx_s = cpool.tile([128, nidx16], I16, name="idxx_s")
            nc.sync.dma_start(idxx_s[:], idx_x_d[:, 0:nidx16])
            mask_s = ld(own_mask_d, [cfg.B, 1])

            q1_tab = dpool.tile([SH, 128], EDT, name="q1_tab")
            q2_tab = dpool.tile([SH, 128], EDT, name="q2_tab")
            kv1_sh = dpool.tile([SH, 256], EDT, name="kv1_sh")
            kv2_sh = dpool.tile([SH, 256], EDT, name="kv2_sh")
            kv1_full = dpool.tile([cfg.N_PAD, 256], EDT, name="kv1_full",
                                  addr_space="Shared")
            kv2_full = dpool.tile([cfg.N_PAD, 256], EDT, name="kv2_full",
                                  addr_space="Shared")
            kv1_hi = dpool.tile([cfg.HALF, 256], EDT, name="kv1_hi")
            kv2_hi = dpool.tile([cfg.HALF, 256], EDT, name="kv2_hi")
            h0_rows = dpool.tile([SH, 64], F32, name="h0_rows")
            h1_rows = dpool.tile([SH, 128], F32, name="h1_rows")
            h2_rows = dpool.tile([SH, 128], F32, name="h2_rows")
            h1T_d = dpool.tile([128, SH], F32, name="h1T_d")
            ar_in = dpool.tile([cfg.B, 2], F32, name="ar_in")
            ar_out = dpool.tile([cfg.B, 2], F32, name="ar_out",
                                addr_space="Shared")

            # ============ encoder + conv1 tables, fused per block ============
            def emit_tab_block(hT_blk, bsl, w_q, b_q_, w_kv, b_kv_, q_tab,
                               kv_sh):
                psq = psA.tile([128, 128], F32, tag="psA", name="ps_q")
                nc.tensor.matmul(psq[:], hT_blk, w_q[:], start=True,
                                 stop=False)
                nc.tensor.matmul(psq[:], ones_s[:], b_q_[:],
                                 start=False, stop=True)
                qr = spool.tile([128, 128], EDT, tag="qr", name="qr")
                nc.scalar.activation(qr[:], psq[:], COPY)
                nc.sync.dma_start(q_tab[bsl, :], qr[:])

                psk = psB.tile([128, 256], F32, tag="psB", name="ps_kv")
                nc.tensor.matmul(psk[:], hT_blk, w_kv[:], start=True,
                                 stop=False)
                nc.tensor.matmul(psk[:], ones_s[:], b_kv_[:],
                                 start=False, stop=True)
                kvr = spool.tile([128, 256], EDT, tag="kvr", name="kvr")
                nc.vector.tensor_copy(kvr[:], psk[:])
                nc.sync.dma_start(kv_sh[bsl, :], kvr[:])

            for b in range(NB):
                bsl = slice(b * 128, (b + 1) * 128)
                xch = wpool.tile([128, 128], F32, tag="xch", name="xch")
                nc.sync.dma_start(xch[:], xT[:, bsl])
                ps1 = psA.tile([32, 128], F32, tag="psA", name="ps_enc1")
                nc.tensor.matmul(ps1[:], w_enc1[:], xch[:], start=True,
                                 stop=True)
                h1p = spool.tile([32, 128], F32, tag="h1p", name="h1p")
                nc.scalar.activation(h1p[:], ps1[:], RELU, bias=b_enc1[:],
                                     scale=1.0)
                ps2 = psA.tile([32, 128], F32, tag="psA", name="ps_enc2")
                nc.tensor.matmul(ps2[:], w_enc2[:], h1p[:], start=True,
                                 stop=True)
                h0b = spool.tile([32, 128], F32, tag="h0b", name="h0b")
                nc.scalar.activation(h0b[:], ps2[:], RELU, bias=b_enc2c[:],
                                     scale=1.0)
                # h0 rows (for the final x1 = h[idx] row gather)
                psr = psA.tile([128, 32], F32, tag="psA", name="ps_h0r")
                nc.tensor.matmul(psr[:], h1p[:], w_enc2[:], start=True,
                                 stop=False)
                nc.tensor.matmul(psr[:], ones_s[:], b_enc2r[:],
                                 start=False, stop=True)
                h0r = spool.tile([128, 64], F32, tag="h0r", name="h0r")
                nc.vector.memset(h0r[:, 32:64], 0.0)
                nc.scalar.activation(h0r[:, 0:32], psr[:], RELU)
                nc.sync.dma_start(h0_rows[bsl, :], h0r[:])
                # conv1 q/kv table rows
                emit_tab_block(h0b[:], bsl, w_q1, b_q1, w_kv1, b_kv1,
                               q1_tab, kv1_sh)
            nc.gpsimd.collective_compute(
                "AllGather", mybir.AluOpType.bypass, replica_groups=RG,
                ins=[kv1_sh.opt()], outs=[kv1_full.opt()])
            nc.sync.dma_start(kv1_hi[:, :], kv1_full[cfg.HALF: cfg.N_PAD, :])

            # ================= conv layers =================
            def emit_conv(q_tab, kv_full, kv_hi_tab, h_rows_out, h_T_out):
                for b in range(NB):
                    T_l, T_h = t_lo[b], t_hi[b]
                    T = T_l + T_h
                    off = blk_off[b]
                    oc = off // 16

                    # zero-offset idx tiles, one per gather
                    kvil = wpool.tile([128, T_l * 8], I16, tag="kvil",
                                      name="kvil")
                    nc.sync.dma_start(kvil[:], kv_idx_d[:, oc: oc + T_l * 8])
                    kvih = wpool.tile([128, T_h * 8], I16, tag="kvih",
                                      name="kvih")
                    nc.sync.dma_start(
                        kvih[:], kv_idx_d[:, oc + T_l * 8: oc + T * 8])
                    qil = wpool.tile([128, T_l * 8], I16, tag="qil",
                                     name="qil")
                    nc.sync.dma_start(qil[:], qi_idx_d[:, oc: oc + T_l * 8])
                    qih = wpool.tile([128, T_h * 8], I16, tag="qih",
                                     name="qih")
                    nc.sync.dma_start(
                        qih[:], qi_idx_d[:, oc + T_l * 8: oc + T * 8])
                    S_b = wpool.tile([128, T * 128], EDT, tag="S_b", name="S_b")
                    nc.sync.dma_start(S_b[:], S_d[:, off: off + T * 128])

                    # zero-offset gather destinations, one per gather
                    kv_lo = wpool.tile([128, T_l, 256], EDT, tag="kv_lo",
                                       name="kv_lo")
                    nc.gpsimd.dma_gather(
                        kv_lo[:], kv_full[0: cfg.HALF, :],
                        kvil[:], T_l * 128, T_l * 128, 256,
                        single_packet=False)
                    kv_hi = wpool.tile([128, T_h, 256], EDT, tag="kv_hi",
                                       name="kv_hi")
                    nc.gpsimd.dma_gather(
                        kv_hi[:], kv_hi_tab[:, :],
                        kvih[:], T_h * 128, T_h * 128, 256,
                        single_packet=False)
                    qi_lo = wpool.tile([128, T_l, 128], EDT, tag="qi_lo",
                                       name="qi_lo")
                    nc.gpsimd.dma_gather(
                        qi_lo[:], q_tab[:, :], qil[:], T_l * 128, T_l * 128,
                        128, single_packet=False)
                    qi_hi = wpool.tile([128, T_h, 128], EDT, tag="qi_hi",
                                       name="qi_hi")
                    nc.gpsimd.dma_gather(
                        qi_hi[:], q_tab[:, :], qih[:], T_h * 128, T_h * 128,
                        128, single_packet=False)

                    prod = w1pool.tile([128, T, 128], EDT, tag="prod",
                                       name="prod")
                    nc.vector.tensor_tensor(prod[:, 0:T_l, :],
                                            qi_lo[:],
                                            kv_lo[:, :, 0:128],
                                            mybir.AluOpType.mult)
                    nc.vector.tensor_tensor(prod[:, T_l:T, :],
                                            qi_hi[:],
                                            kv_hi[:, :, 0:128],
                                            mybir.AluOpType.mult)
                    l_t = spool.tile([128, T * 4], F32, tag="l_t", name="l_t")
                    nc.vector.tensor_reduce(
                        l_t[:].rearrange("p (t h) -> p t h", h=4),
                        prod[:].rearrange("p t (h j) -> p t h j", h=4, j=32),
                        mybir.AxisListType.X, mybir.AluOpType.add)

                    rhs = wpool.tile([128, T, 132], EDT, tag="rhs", name="rhs")
                    nc.scalar.activation(
                        rhs[:, :, 0:4],
                        l_t[:].rearrange("p (t h) -> p t h", h=4),
                        EXP, scale=float(cfg.SCALE))
                    aw = w1pool.tile([128, T, 128], EDT, tag="aw", name="aw")
                    nc.scalar.activation(
                        aw[:].rearrange("p t (h j) -> p t h j", h=4, j=32),
                        l_t[:].rearrange("p (t h) -> p t h", h=4)
                            .unsqueeze(-1).broadcast_to([128, T, 4, 32]),
                        EXP, scale=float(cfg.SCALE))
                    nc.vector.tensor_tensor(rhs[:, 0:T_l, 4:132],
                                            kv_lo[:, :, 128:256],
                                            aw[:, 0:T_l, :],
                                            mybir.AluOpType.mult)
                    nc.vector.tensor_tensor(rhs[:, T_l:T, 4:132],
                                            kv_hi[:, :, 128:256],
                                            aw[:, T_l:T, :],
                                            mybir.AluOpType.mult)

                    sc_ps = psB.tile([128, 132], F32, tag="psB", name="sc_ps")
                    for t in range(T):
                        nc.tensor.matmul(
                            sc_ps[:], S_b[:, t * 128: (t + 1) * 128],
                            rhs[:, t, :], start=(t == 0), stop=(t == T - 1))

                    den = spool.tile([128, 4], F32, tag="den", name="den")
                    nc.vector.tensor_scalar_add(den[:], sc_ps[:, 0:4],
                                                float(cfg.EPS))
                    rec = spool.tile([128, 4], F32, tag="rec", name="rec")
                    nc.vector.reciprocal(rec[:], den[:])
                    h_blk = spool.tile([128, 128], F32, tag="h_blk",
                                       name="h_blk")
                    for h in range(4):
                        nc.scalar.activation(
                            h_blk[:, h * 32: (h + 1) * 32],
                            sc_ps[:, 4 + h * 32: 4 + (h + 1) * 32],
                            RELU, scale=rec[:, h: h + 1])
                    bsl = slice(b * 128, (b + 1) * 128)
                    nc.sync.dma_start(h_rows_out[bsl, :], h_blk[:])
                    if h_T_out is not None:
                        tr_ps = psC.tile([128, 128], F32, tag="psC",
                                         name="tr_ps")
                        nc.tensor.transpose(tr_ps[:], h_blk[:], id_s[:])
                        h1tb = spool.tile([128, 128], F32, tag="h1tb",
                                          name="h1tb")
                        nc.scalar.activation(h1tb[:], tr_ps[:], COPY)
                        nc.sync.dma_start(h_T_out[:, bsl], h1tb[:])

            emit_conv(q1_tab, kv1_full, kv1_hi, h1_rows, h1T_d)

            # conv2 tables (h1T streamed back from DRAM per block)
            for b in range(NB):
                bsl = slice(b * 128, (b + 1) * 128)
                h1c = wpool.tile([128, 128], F32, tag="h1c", name="h1c")
                nc.sync.dma_start(h1c[:], h1T_d[:, bsl])
                emit_tab_block(h1c[:], bsl, w_q2, b_q2, w_kv2, b_kv2,
                               q2_tab, kv2_sh)
            nc.gpsimd.collective_compute(
                "AllGather", mybir.AluOpType.bypass, replica_groups=RG,
                ins=[kv2_sh.opt()], outs=[kv2_full.opt()])
            nc.sync.dma_start(kv2_hi[:, :], kv2_full[cfg.HALF: cfg.N_PAD, :])

            emit_conv(q2_tab, kv2_full, kv2_hi, h2_rows, None)

            # ================= Q head =================
            def gather_xT(tab, width):
                g = spool.tile([128, 1, width], F32, tag="gx", name="gx")
                nc.gpsimd.dma_gather(g[:], tab[:, :], idxx_s[:],
                                     cfg.B, cfg.B, width)
                tp = psC.tile([128, 128], F32, tag="psC", name="tp_x")
                nc.tensor.transpose(tp[0:width, 0: cfg.B], g[0: cfg.B, 0, :],
                                    id_s[0: cfg.B, 0: cfg.B])
                xt = spool.tile([128, cfg.B], F32, tag="xt", name="xt")
                nc.scalar.activation(xt[0:width, :], tp[0:width, 0: cfg.B],
                                     COPY)
                return xt

            x1t = gather_xT(h0_rows, 64)
            x2t = gather_xT(h1_rows, 128)
            x3t = gather_xT(h2_rows, 128)

            zh_ps = psA.tile([128, cfg.B], F32, tag="psA", name="zh_ps")
            nc.tensor.matmul(zh_ps[:], w_qha[:], x1t[0:32, :],
                             start=True, stop=False)
            nc.tensor.matmul(zh_ps[:], w_qhb[:], x2t[0:128, :],
                             start=False, stop=False)
            nc.tensor.matmul(zh_ps[:], w_qhc[:], x3t[0:128, :],
                             start=False, stop=True)
            zh = spool.tile([128, cfg.B], F32, tag="zh", name="zh")
            nc.scalar.activation(zh[:], zh_ps[:], RELU, bias=b_qh[:],
                                 scale=1.0)
            o_ps = psB.tile([cfg.B, 2], F32, tag="psB", name="o_ps")
            nc.tensor.matmul(o_ps[:], zh[:], w_qh2[:], start=True, stop=False)
            nc.tensor.matmul(o_ps[:], ones_s[:, 0: cfg.B], b_qh2[:],
                             start=False, stop=True)
            ob = spool.tile([cfg.B, 2], F32, tag="ob", name="ob")
            nc.vector.tensor_scalar_mul(ob[:], o_ps[:], mask_s[:])
            nc.sync.dma_start(ar_in[:, :], ob[:])
            nc.gpsimd.collective_compute(
                "AllReduce", mybir.AluOpType.add, replica_groups=RG,
                ins=[ar_in.opt()], outs=[ar_out.opt()])
            nc.sync.dma_start(out_d[:, :], ar_out[:, :])

    nc.compile()
    return nc


# --------------------------------------------------------------------------
# entry point
# --------------------------------------------------------------------------

_trace_flag = {"trace": False}
_last = {}


def _run(inputs, cfg=None):
    cfg = cfg or Cfg()
    in_maps, t_lo, t_hi, blk_off, slots = _prep_inputs(cfg, inputs)
    key = (slots, tuple(t_lo), tuple(t_hi), cfg.edge_bf16)
    if _last.get("key") != key:
        _last["nc"] = build_program(cfg, t_lo, t_hi, blk_off, slots)
        _last["key"] = key
    nc = _last["nc"]
    res = bass_utils.run_bass_kernel_spmd(
        nc, in_maps, core_ids=list(range(N_CORES)),
        trace=_trace_flag["trace"])
    _last["res"] = res
    return res.results[0]["out"].astype(np.float32)


def kernel(**inputs):
    return _run(inputs)



# revision 2
# speedup vs baseline: 1.4729x; 1.4729x over previous
"""Trainium2 Bass kernel for DGNRNetwork (2-layer TransformerConv GNN + MLPs).

Strategy (8 NeuronCores, graph/data parallel):
  - Nodes padded to N_PAD=50176 and sharded by contiguous range: core c owns
    nodes [c*6272, (c+1)*6272), i.e. 49 blocks of 128 dst nodes per core.
  - Edges are partitioned by dst shard on host and laid out in chunked order
    (chunk of consecutive dst blocks, then src-half, then block, then src),
    padded so every (block, src-half) group is a whole number of 128-edge
    tiles (tile counts uniform across cores -> one SPMD program).
  - k||v rows are fetched with ONE indirect DMA (dma_gather) per
    (chunk, src-half) - the gather's GpSimd descriptor generation cost is
    ~8ns/row, so few big gathers instead of many small ones.
  - q rows are never gathered: within a dst block the q rows come from only
    128 nodes, so qi = S_T_tile @ Q_blk is a TensorE matmul with a
    host-precomputed one-hot S_T (S_T[d, e] = 1 iff edge e's dst is d).
  - Per-edge attention logits on Vector (reading qi straight from PSUM);
    exp on Scalar; the segment-softmax denominator and the weighted sum over
    incoming edges are ONE accumulated TensorE matmul with the one-hot
    scatter matrix S (S[e, d] = edge e's dst is block-node d). Padding edges
    have all-zero S rows so they drop out.
  - Small weights are replicated; k/v row tables are exchanged with an
    AllGather between the conv layers; the tiny Q-head is computed
    redundantly and combined with a masked AllReduce.
"""

import sys

sys.path.insert(0, "/opt/trn_rl_repo")

import numpy as np
import ml_dtypes

import concourse.bacc as bacc
import concourse.bass as bass
import concourse.mybir as mybir
import concourse.tile as tile
from concourse import bass_utils, library_config

F32 = mybir.dt.float32
BF16 = mybir.dt.bfloat16
I16 = mybir.dt.int16

N_CORES = 8


class Cfg:
    def __init__(self, n_nodes=50000, nblk=49, b=64, edge_bf16=True,
                 chunk_tiles=104, qi_group=6):
        self.N = n_nodes
        self.NBLK = nblk                 # dst blocks per core
        self.SHARD = nblk * 128          # nodes per core
        self.N_PAD = 8 * self.SHARD
        self.HALF = self.N_PAD // 2      # kv table split (int16 gather idx)
        self.B = b                       # batch (selected nodes)
        self.F_IN = 128
        self.H = 32
        self.HD = 128
        self.HEADS = 4
        self.EPS = 1e-16
        self.SCALE = 1.0 / np.sqrt(32.0)
        self.edge_bf16 = edge_bf16
        self.EDT = BF16 if edge_bf16 else F32
        self.EDT_NP = ml_dtypes.bfloat16 if edge_bf16 else np.float32
        self.CHUNK_TILES = chunk_tiles   # max 128-edge tiles per kv chunk
        self.QI_GROUP = qi_group         # tiles per qi PSUM group
        assert self.N <= self.N_PAD and self.HALF < 32768


# --------------------------------------------------------------------------
# host-side preprocessing
# --------------------------------------------------------------------------


def _wrap16(values, slots):
    """dma_gather idx layout: idx i lives at [i % 16, i // 16], replicated
    across the eight 16-partition groups."""
    arr = np.zeros((16, slots // 16), dtype=np.int16)
    arr[np.arange(len(values)) % 16, np.arange(len(values)) // 16] = values
    return np.tile(arr, (8, 1))


def _plan_chunks(cfg, t_lo, t_hi):
    """Greedily group consecutive blocks into chunks of <= CHUNK_TILES tiles.

    Returns chunks: list of dicts with
      blocks: list of block ids
      t_lo_sum / t_hi_sum: tiles in the chunk's lo / hi region
      tile0: global tile index where this chunk starts
      runs[b]: per block, two (region_tile_off_in_chunk, ntiles) runs
    and the global tile layout [chunk0: lo tiles (block major) | hi tiles
    (block major)] [chunk1: ...].
    """
    chunks = []
    b = 0
    tile0 = 0
    while b < cfg.NBLK:
        blocks = []
        tl = th = 0
        while b < cfg.NBLK:
            need = t_lo[b] + t_hi[b]
            if blocks and tl + th + need > cfg.CHUNK_TILES:
                break
            blocks.append(b)
            tl += t_lo[b]
            th += t_hi[b]
            b += 1
        runs = {}
        lo_off = 0
        hi_off = tl
        for blk in blocks:
            runs[blk] = ((lo_off, t_lo[blk]), (hi_off, t_hi[blk]))
            lo_off += t_lo[blk]
            hi_off += t_hi[blk]
        chunks.append(dict(blocks=blocks, t_lo_sum=tl, t_hi_sum=th,
                           tile0=tile0, runs=runs))
        tile0 += tl + th
    return chunks, tile0


def _prep_edges(cfg, edge_index):
    src = np.ascontiguousarray(edge_index[0]).astype(np.int64)
    dst = np.ascontiguousarray(edge_index[1]).astype(np.int64)
    core = dst // cfg.SHARD
    blk = (dst % cfg.SHARD) // 128
    hi = (src >= cfg.HALF).astype(np.int64)

    cnt = np.zeros((N_CORES, cfg.NBLK, 2), np.int64)
    np.add.at(cnt, (core, blk, hi), 1)
    t_lo = np.maximum(1, (cnt[:, :, 0].max(0) + 127) // 128)  # [NBLK]
    t_hi = np.maximum(1, (cnt[:, :, 1].max(0) + 127) // 128)

    chunks, total_tiles = _plan_chunks(cfg, t_lo.tolist(), t_hi.tolist())
    slots = total_tiles * 128
    assert slots % 16 == 0

    # global slot base for each (blk, hi) group
    grp_base = np.zeros((cfg.NBLK, 2), np.int64)
    for ch in chunks:
        for blk_id in ch["blocks"]:
            (lo_off, _), (hi_off, _) = ch["runs"][blk_id]
            grp_base[blk_id, 0] = (ch["tile0"] + lo_off) * 128
            grp_base[blk_id, 1] = (ch["tile0"] + hi_off) * 128

    chunk_of_blk = np.zeros(cfg.NBLK, np.int64)
    for ci, ch in enumerate(chunks):
        for blk_id in ch["blocks"]:
            chunk_of_blk[blk_id] = ci

    order = np.lexsort((src, blk, hi, chunk_of_blk[blk], core))
    s_src, s_dst, s_core, s_blk, s_hi = (
        src[order], dst[order], core[order], blk[order], hi[order])

    per_core = []
    for c in range(N_CORES):
        m = s_core == c
        csrc, cdst, cblk, chi = s_src[m], s_dst[m], s_blk[m], s_hi[m]
        # composite key non-decreasing under the sort above
        key = (chunk_of_blk[cblk] * 2 + chi) * 64 + cblk
        kcounts = np.bincount(key, minlength=(len(chunks) * 2) * 64)
        kstarts = np.zeros_like(kcounts)
        kstarts[1:] = np.cumsum(kcounts)[:-1]
        rank = np.arange(len(key)) - kstarts[key]
        slot = grp_base[cblk, chi] + rank

        kv_val = np.where(chi == 1, csrc - cfg.HALF, csrc)
        kv_idx = np.zeros(slots, np.int64)
        kv_idx[slot] = kv_val

        S = np.zeros((128, slots), cfg.EDT_NP)
        scol = (slot // 128) * 128 + (cdst % 128)
        S[slot % 128, scol] = 1.0
        ST = np.zeros((128, slots), cfg.EDT_NP)
        stcol = (slot // 128) * 128 + (slot % 128)
        ST[cdst % 128, stcol] = 1.0

        per_core.append(dict(kv_idx=_wrap16(kv_idx, slots), S=S, ST=ST))
    return per_core, t_lo.tolist(), t_hi.tolist(), chunks, slots


def _prep_inputs(cfg, inputs):
    x = np.asarray(inputs["x"], np.float32)
    idx = np.asarray(inputs["idx"]).astype(np.int64)
    f32 = lambda k: np.ascontiguousarray(np.asarray(inputs[k], np.float32))

    xp = np.zeros((cfg.N_PAD, cfg.F_IN), np.float32)
    xp[: cfg.N] = x

    per_core_e, t_lo, t_hi, chunks, slots = _prep_edges(cfg, inputs["edge_index"])

    wkv1 = np.ascontiguousarray(
        np.concatenate([f32("c1_wk"), f32("c1_wv")], axis=1))     # [32,256]
    bkv1 = np.ascontiguousarray(
        np.concatenate([f32("c1_bk"), f32("c1_bv")])[None, :])    # [1,256]
    wkv2 = np.ascontiguousarray(
        np.concatenate([f32("c2_wk"), f32("c2_wv")], axis=1))     # [128,256]
    bkv2 = np.ascontiguousarray(
        np.concatenate([f32("c2_bk"), f32("c2_bv")])[None, :])
    qw1 = f32("q_w1")                                              # [288,128]
    bpad = ((cfg.B + 127) // 128) * 128

    in_maps = []
    for c in range(N_CORES):
        shard = slice(c * cfg.SHARD, (c + 1) * cfg.SHARD)
        own = (idx // cfg.SHARD) == c
        idx_loc = np.where(own, idx - c * cfg.SHARD, 0)
        im = dict(
            xT=np.ascontiguousarray(xp[shard].T),          # [128, SHARD]
            enc_w1=f32("enc_w1"),
            enc_b1=f32("enc_b1").reshape(32, 1),
            enc_w2=f32("enc_w2"),
            enc_b2c=f32("enc_b2").reshape(32, 1),
            enc_b2r=f32("enc_b2").reshape(1, 32),
            wq1=f32("c1_wq"), bq1=np.ascontiguousarray(f32("c1_bq")[None, :]),
            wkv1=wkv1, bkv1=bkv1,
            wq2=f32("c2_wq"), bq2=np.ascontiguousarray(f32("c2_bq")[None, :]),
            wkv2=wkv2, bkv2=bkv2,
            qw1a=np.ascontiguousarray(qw1[0:32]),
            qw1b=np.ascontiguousarray(qw1[32:160]),
            qw1c=np.ascontiguousarray(qw1[160:288]),
            qb1=f32("q_b1").reshape(128, 1),
            qw2=f32("q_w2"),
            qb2=f32("q_b2").reshape(1, 2),
            ones128=np.ones((1, 128), np.float32),
            id128=np.eye(128, dtype=np.float32),
            kv_idx=per_core_e[c]["kv_idx"],
            S_all=per_core_e[c]["S"],
            ST_all=per_core_e[c]["ST"],
            idx_x=_wrap16(idx_loc, bpad),
            own_mask=own.astype(np.float32).reshape(cfg.B, 1),
        )
        in_maps.append(im)
    return in_maps, t_lo, t_hi, chunks, slots


# --------------------------------------------------------------------------
# device program
# --------------------------------------------------------------------------


def build_program(cfg, t_lo, t_hi, chunks, slots):
    nc = bacc.Bacc("TRN2", target_bir_lowering=False, debug=False,
                   num_devices=N_CORES)
    EDT = cfg.EDT
    NB, SH = cfg.NBLK, cfg.SHARD
    RG = [list(range(N_CORES))]
    RELU = mybir.ActivationFunctionType.Relu
    COPY = mybir.ActivationFunctionType.Copy
    EXP = mybir.ActivationFunctionType.Exp

    def din(name, shape, dt=F32):
        return nc.dram_tensor(name, list(shape), dt, kind="ExternalInput").ap()

    xT = din("xT", [128, SH])
    enc_w1 = din("enc_w1", [128, 32]); enc_b1 = din("enc_b1", [32, 1])
    enc_w2 = din("enc_w2", [32, 32]); enc_b2c = din("enc_b2c", [32, 1])
    enc_b2r = din("enc_b2r", [1, 32])
    wq1 = din("wq1", [32, 128]); bq1 = din("bq1", [1, 128])
    wkv1 = din("wkv1", [32, 256]); bkv1 = din("bkv1", [1, 256])
    wq2 = din("wq2", [128, 128]); bq2 = din("bq2", [1, 128])
    wkv2 = din("wkv2", [128, 256]); bkv2 = din("bkv2", [1, 256])
    qw1a = din("qw1a", [32, 128]); qw1b = din("qw1b", [128, 128])
    qw1c = din("qw1c", [128, 128]); qb1 = din("qb1", [128, 1])
    qw2 = din("qw2", [128, 2]); qb2 = din("qb2", [1, 2])
    ones128 = din("ones128", [1, 128]); id128 = din("id128", [128, 128])
    kv_idx_d = din("kv_idx", [128, slots // 16], I16)
    S_d = din("S_all", [128, slots], EDT)
    ST_d = din("ST_all", [128, slots], EDT)
    bpad = ((cfg.B + 127) // 128) * 128
    idx_x_d = din("idx_x", [128, bpad // 16], I16)
    own_mask_d = din("own_mask", [cfg.B, 1])
    out_d = nc.dram_tensor("out", [cfg.B, 2], F32, kind="ExternalOutput").ap()

    with tile.TileContext(nc) as tc:
        with (
            tc.tile_pool(name="const", bufs=1) as cpool,
            tc.tile_pool(name="kvch", bufs=2) as kvpool,
            tc.tile_pool(name="work", bufs=2) as wpool,
            tc.tile_pool(name="small", bufs=3) as spool,
            tc.tile_pool(name="psA", bufs=2, space="PSUM") as psA,
            tc.tile_pool(name="psB", bufs=2, space="PSUM") as psB,
            tc.tile_pool(name="psC", bufs=2, space="PSUM") as psC,
            tc.tile_pool(name="dram", bufs=1, space="DRAM") as dpool,
        ):
            nc.gpsimd.load_library(library_config.mlp)

            def ld(ap, shape, dt=F32, nm=None):
                t = cpool.tile(shape, dt, name=nm or ("ld_" + ap.tensor.name))
                nc.sync.dma_start(t[:], ap[:])
                return t

            w_enc1 = ld(enc_w1, [128, 32]); b_enc1 = ld(enc_b1, [32, 1])
            w_enc2 = ld(enc_w2, [32, 32]); b_enc2c = ld(enc_b2c, [32, 1])
            b_enc2r = ld(enc_b2r, [1, 32])
            w_q1 = ld(wq1, [32, 128]); b_q1 = ld(bq1, [1, 128])
            w_kv1 = ld(wkv1, [32, 256]); b_kv1 = ld(bkv1, [1, 256])
            w_q2 = ld(wq2, [128, 128]); b_q2 = ld(bq2, [1, 128])
            w_kv2 = ld(wkv2, [128, 256]); b_kv2 = ld(bkv2, [1, 256])
            w_qha = ld(qw1a, [32, 128]); w_qhb = ld(qw1b, [128, 128])
            w_qhc = ld(qw1c, [128, 128])
            b_qh = ld(qb1, [128, 1]); w_qh2 = ld(qw2, [128, 2])
            b_qh2 = ld(qb2, [1, 2])
            ones_s = ld(ones128, [1, 128]); id_s = ld(id128, [128, 128])
            nidx16 = (cfg.B + 15) // 16
            idxx_s = cpool.tile([128, nidx16], I16, name="idxx_s")
            nc.sync.dma_start(idxx_s[:], idx_x_d[:, 0:nidx16])
            mask_s = ld(own_mask_d, [cfg.B, 1])
            kvidx_s = cpool.tile([128, slots // 16], I16, name="kvidx_s")
            nc.sync.dma_start(kvidx_s[:], kv_idx_d[:])

            q1_tab = dpool.tile([SH, 128], EDT, name="q1_tab")
            q2_tab = dpool.tile([SH, 128], EDT, name="q2_tab")
            kv1_sh = dpool.tile([SH, 256], EDT, name="kv1_sh")
            kv2_sh = dpool.tile([SH, 256], EDT, name="kv2_sh")
            kv1_full = dpool.tile([cfg.N_PAD, 256], EDT, name="kv1_full",
                                  addr_space="Shared")
            kv2_full = dpool.tile([cfg.N_PAD, 256], EDT, name="kv2_full",
                                  addr_space="Shared")
            h0_rows = dpool.tile([SH, 64], F32, name="h0_rows")
            h1_rows = dpool.tile([SH, 128], F32, name="h1_rows")
            h2_rows = dpool.tile([SH, 128], F32, name="h2_rows")
            h1T_d = dpool.tile([128, SH], F32, name="h1T_d")
            ar_in = dpool.tile([cfg.B, 2], F32, name="ar_in")
            ar_out = dpool.tile([cfg.B, 2], F32, name="ar_out",
                                addr_space="Shared")

            # ============ encoder + conv1 tables, fused per block ============
            def emit_tab_block(hT_blk, bsl, w_q, b_q_, w_kv, b_kv_, q_tab,
                               kv_sh):
                psq = psA.tile([128, 128], F32, tag="psA", name="ps_q")
                nc.tensor.matmul(psq[:], hT_blk, w_q[:], start=True,
                                 stop=False)
                nc.tensor.matmul(psq[:], ones_s[:], b_q_[:],
                                 start=False, stop=True)
                qr = spool.tile([128, 128], EDT, tag="qr", name="qr")
                nc.scalar.activation(qr[:], psq[:], COPY)
                nc.sync.dma_start(q_tab[bsl, :], qr[:])

                psk = psB.tile([128, 256], F32, tag="psB", name="ps_kv")
                nc.tensor.matmul(psk[:], hT_blk, w_kv[:], start=True,
                                 stop=False)
                nc.tensor.matmul(psk[:], ones_s[:], b_kv_[:],
                                 start=False, stop=True)
                kvr = spool.tile([128, 256], EDT, tag="kvr", name="kvr")
                nc.vector.tensor_copy(kvr[:], psk[:])
                nc.sync.dma_start(kv_sh[bsl, :], kvr[:])

            for b in range(NB):
                bsl = slice(b * 128, (b + 1) * 128)
                xch = wpool.tile([128, 128], F32, tag="xch", name="xch")
                nc.sync.dma_start(xch[:], xT[:, bsl])
                ps1 = psA.tile([32, 128], F32, tag="psA", name="ps_enc1")
                nc.tensor.matmul(ps1[:], w_enc1[:], xch[:], start=True,
                                 stop=True)
                h1p = spool.tile([32, 128], F32, tag="h1p", name="h1p")
                nc.scalar.activation(h1p[:], ps1[:], RELU, bias=b_enc1[:],
                                     scale=1.0)
                ps2 = psA.tile([32, 128], F32, tag="psA", name="ps_enc2")
                nc.tensor.matmul(ps2[:], w_enc2[:], h1p[:], start=True,
                                 stop=True)
                h0b = spool.tile([32, 128], F32, tag="h0b", name="h0b")
                nc.scalar.activation(h0b[:], ps2[:], RELU, bias=b_enc2c[:],
                                     scale=1.0)
                # h0 rows (for the final x1 = h[idx] row gather)
                psr = psA.tile([128, 32], F32, tag="psA", name="ps_h0r")
                nc.tensor.matmul(psr[:], h1p[:], w_enc2[:], start=True,
                                 stop=False)
                nc.tensor.matmul(psr[:], ones_s[:], b_enc2r[:],
                                 start=False, stop=True)
                h0r = spool.tile([128, 64], F32, tag="h0r", name="h0r")
                nc.vector.memset(h0r[:, 32:64], 0.0)
                nc.scalar.activation(h0r[:, 0:32], psr[:], RELU)
                nc.sync.dma_start(h0_rows[bsl, :], h0r[:])
                # conv1 q/kv table rows
                emit_tab_block(h0b[:], bsl, w_q1, b_q1, w_kv1, b_kv1,
                               q1_tab, kv1_sh)
            nc.gpsimd.collective_compute(
                "AllGather", mybir.AluOpType.bypass, replica_groups=RG,
                ins=[kv1_sh.opt()], outs=[kv1_full.opt()])

            # ================= conv layers =================
            def emit_conv(q_tab, kv_full, h_rows_out, h_T_out):
                for ch in chunks:
                    TL, TH = ch["t_lo_sum"], ch["t_hi_sum"]
                    TC = TL + TH
                    tile0 = ch["tile0"]
                    oc = tile0 * 8  # idx cols (16 idx per col)

                    kv_ch = kvpool.tile([128, TC, 256], EDT, tag="kv_ch",
                                        name="kv_ch")
                    nc.gpsimd.dma_gather(
                        kv_ch[:, 0:TL, :], kv_full[0: cfg.HALF, :],
                        kvidx_s[:, oc: oc + TL * 8], TL * 128, TL * 128, 256,
                        single_packet=False)
                    nc.gpsimd.dma_gather(
                        kv_ch[:, TL:TC, :], kv_full[cfg.HALF: cfg.N_PAD, :],
                        kvidx_s[:, oc + TL * 8: oc + TC * 8], TH * 128,
                        TH * 128, 256, single_packet=False)

                    for b in ch["blocks"]:
                        runs = [r for r in ch["runs"][b] if r[1] > 0]
                        T = sum(r[1] for r in runs)
                        # block's S / S_T tiles (per run, from global layout)
                        s_tiles = []
                        st_tiles = []
                        for off, nt in runs:
                            g0 = (tile0 + off) * 128
                            s_t = wpool.tile([128, nt * 128], EDT, tag="S_b",
                                             name="S_b")
                            nc.sync.dma_start(s_t[:], S_d[:, g0: g0 + nt * 128])
                            s_tiles.append(s_t)
                            st_t = wpool.tile([128, nt * 128], EDT,
                                              tag="ST_b", name="ST_b")
                            nc.sync.dma_start(st_t[:],
                                              ST_d[:, g0: g0 + nt * 128])
                            st_tiles.append(st_t)
                        bsl = slice(b * 128, (b + 1) * 128)
                        q_blk = wpool.tile([128, 128], EDT, tag="q_blk",
                                           name="q_blk")
                        nc.sync.dma_start(q_blk[:], q_tab[bsl, :])

                        l_t = spool.tile([128, T * 4], F32, tag="l_t",
                                         name="l_t")
                        # qi via S_T @ Q_blk, prod+reduce per <=QI_GROUP tiles
                        lpos = 0
                        for (off, nt), st_t in zip(runs, st_tiles):
                            for g0 in range(0, nt, cfg.QI_GROUP):
                                g = min(cfg.QI_GROUP, nt - g0)
                                qi_ps = psA.tile([128, g, 128], F32,
                                                 tag="psA", name="qi_ps")
                                for i in range(g):
                                    c0 = (g0 + i) * 128
                                    nc.tensor.matmul(
                                        qi_ps[:, i, :],
                                        st_t[:, c0: c0 + 128], q_blk[:],
                                        start=True, stop=True)
                                prod = spool.tile([128, g, 128], EDT,
                                                  tag="prod", name="prod")
                                nc.vector.tensor_tensor(
                                    prod[:],
                                    qi_ps[:],
                                    kv_ch[:, off + g0: off + g0 + g, 0:128],
                                    mybir.AluOpType.mult)
                                nc.vector.tensor_reduce(
                                    l_t[:, lpos: lpos + g * 4].rearrange(
                                        "p (t h) -> p t h", h=4),
                                    prod[:].rearrange(
                                        "p t (h j) -> p t h j", h=4, j=32),
                                    mybir.AxisListType.X, mybir.AluOpType.add)
                                lpos += g * 4

                        rhs = wpool.tile([128, T, 132], EDT, tag="rhs",
                                         name="rhs")
                        nc.scalar.activation(
                            rhs[:, :, 0:4],
                            l_t[:].rearrange("p (t h) -> p t h", h=4),
                            EXP, scale=float(cfg.SCALE))
                        aw = wpool.tile([128, T, 128], EDT, tag="aw",
                                        name="aw")
                        nc.scalar.activation(
                            aw[:].rearrange("p t (h j) -> p t h j", h=4, j=32),
                            l_t[:].rearrange("p (t h) -> p t h", h=4)
                                .unsqueeze(-1).broadcast_to([128, T, 4, 32]),
                            EXP, scale=float(cfg.SCALE))
                        tpos = 0
                        for off, nt in runs:
                            nc.vector.tensor_tensor(
                                rhs[:, tpos: tpos + nt, 4:132],
                                kv_ch[:, off: off + nt, 128:256],
                                aw[:, tpos: tpos + nt, :],
                                mybir.AluOpType.mult)
                            tpos += nt

                        sc_ps = psB.tile([128, 132], F32, tag="psB",
                                         name="sc_ps")
                        t_i = 0
                        for s_t, (off, nt) in zip(s_tiles, runs):
                            for i in range(nt):
                                nc.tensor.matmul(
                                    sc_ps[:], s_t[:, i * 128: (i + 1) * 128],
                                    rhs[:, t_i, :], start=(t_i == 0),
                                    stop=(t_i == T - 1))
                                t_i += 1

                        den = spool.tile([128, 4], F32, tag="den", name="den")
                        nc.vector.tensor_scalar_add(den[:], sc_ps[:, 0:4],
                                                    float(cfg.EPS))
                        rec = spool.tile([128, 4], F32, tag="rec", name="rec")
                        nc.vector.reciprocal(rec[:], den[:])
                        h_blk = spool.tile([128, 128], F32, tag="h_blk",
                                           name="h_blk")
                        for h in range(4):
                            nc.scalar.activation(
                                h_blk[:, h * 32: (h + 1) * 32],
                                sc_ps[:, 4 + h * 32: 4 + (h + 1) * 32],
                                RELU, scale=rec[:, h: h + 1])
                        nc.sync.dma_start(h_rows_out[bsl, :], h_blk[:])
                        if h_T_out is not None:
                            tr_ps = psC.tile([128, 128], F32, tag="psC",
                                             name="tr_ps")
                            nc.tensor.transpose(tr_ps[:], h_blk[:], id_s[:])
                            h1tb = spool.tile([128, 128], F32, tag="h1tb",
                                              name="h1tb")
                            nc.scalar.activation(h1tb[:], tr_ps[:], COPY)
                            nc.sync.dma_start(h_T_out[:, bsl], h1tb[:])

            emit_conv(q1_tab, kv1_full, h1_rows, h1T_d)

            # conv2 tables (h1T streamed back from DRAM per block)
            for b in range(NB):
                bsl = slice(b * 128, (b + 1) * 128)
                h1c = wpool.tile([128, 128], F32, tag="h1c", name="h1c")
                nc.sync.dma_start(h1c[:], h1T_d[:, bsl])
                emit_tab_block(h1c[:], bsl, w_q2, b_q2, w_kv2, b_kv2,
                               q2_tab, kv2_sh)
            nc.gpsimd.collective_compute(
                "AllGather", mybir.AluOpType.bypass, replica_groups=RG,
                ins=[kv2_sh.opt()], outs=[kv2_full.opt()])

            emit_conv(q2_tab, kv2_full, h2_rows, None)

            # ================= Q head =================
            def gather_xT(tab, width):
                g = spool.tile([128, 1, width], F32, tag="gx", name="gx")
                nc.gpsimd.dma_gather(g[:], tab[:, :], idxx_s[:],
                                     cfg.B, cfg.B, width)
                tp = psC.tile([128, 128], F32, tag="psC", name="tp_x")
                nc.tensor.transpose(tp[0:width, 0: cfg.B], g[0: cfg.B, 0, :],
                                    id_s[0: cfg.B, 0: cfg.B])
                xt = spool.tile([128, cfg.B], F32, tag="xt", name="xt")
                nc.scalar.activation(xt[0:width, :], tp[0:width, 0: cfg.B],
                                     COPY)
                return xt

            x1t = gather_xT(h0_rows, 64)
            x2t = gather_xT(h1_rows, 128)
            x3t = gather_xT(h2_rows, 128)

            zh_ps = psA.tile([128, cfg.B], F32, tag="psA", name="zh_ps")
            nc.tensor.matmul(zh_ps[:], w_qha[:], x1t[0:32, :],
                             start=True, stop=False)
            nc.tensor.matmul(zh_ps[:], w_qhb[:], x2t[0:128, :],
                             start=False, stop=False)
            nc.tensor.matmul(zh_ps[:], w_qhc[:], x3t[0:128, :],
                             start=False, stop=True)
            zh = spool.tile([128, cfg.B], F32, tag="zh", name="zh")
            nc.scalar.activation(zh[:], zh_ps[:], RELU, bias=b_qh[:],
                                 scale=1.0)
            o_ps = psB.tile([cfg.B, 2], F32, tag="psB", name="o_ps")
            nc.tensor.matmul(o_ps[:], zh[:], w_qh2[:], start=True, stop=False)
            nc.tensor.matmul(o_ps[:], ones_s[:, 0: cfg.B], b_qh2[:],
                             start=False, stop=True)
            ob = spool.tile([cfg.B, 2], F32, tag="ob", name="ob")
            nc.vector.tensor_scalar_mul(ob[:], o_ps[:], mask_s[:])
            nc.sync.dma_start(ar_in[:, :], ob[:])
            nc.gpsimd.collective_compute(
                "AllReduce", mybir.AluOpType.add, replica_groups=RG,
                ins=[ar_in.opt()], outs=[ar_out.opt()])
            nc.sync.dma_start(out_d[:, :], ar_out[:, :])

    nc.compile()
    return nc


# --------------------------------------------------------------------------
# entry point
# --------------------------------------------------------------------------

_trace_flag = {"trace": False}
_last = {}


def _chunk_key(chunks):
    return tuple((tuple(ch["blocks"]), ch["t_lo_sum"], ch["t_hi_sum"])
                 for ch in chunks)


def _run(inputs, cfg=None):
    cfg = cfg or Cfg()
    in_maps, t_lo, t_hi, chunks, slots = _prep_inputs(cfg, inputs)
    key = (slots, tuple(t_lo), tuple(t_hi), _chunk_key(chunks), cfg.edge_bf16)
    if _last.get("key") != key:
        _last["nc"] = build_program(cfg, t_lo, t_hi, chunks, slots)
        _last["key"] = key
    nc = _last["nc"]
    res = bass_utils.run_bass_kernel_spmd(
        nc, in_maps, core_ids=list(range(N_CORES)),
        trace=_trace_flag["trace"])
    _last["res"] = res
    return res.results[0]["out"].astype(np.float32)


def kernel(**inputs):
    return _run(inputs)


# revision 8
# speedup vs baseline: 1.8130x; 1.2310x over previous
"""Trainium2 Bass kernel for DGNRNetwork (2-layer TransformerConv GNN + MLPs).

Strategy (8 NeuronCores, graph/data parallel):
  - Nodes padded to N_PAD=50176 and sharded by contiguous range: core c owns
    nodes [c*6272, (c+1)*6272), i.e. 49 blocks of 128 dst nodes per core.
  - Edges are partitioned by dst shard on host and laid out in chunked order
    (chunk of consecutive dst blocks, then src-half, then block, then src),
    padded so every (block, src-half) group is a whole number of 128-edge
    tiles (tile counts uniform across cores -> one SPMD program).
  - k||v rows are fetched with ONE indirect DMA (dma_gather) per
    (chunk, src-half) - the gather's GpSimd descriptor generation cost is
    ~8ns/row, so few big gathers instead of many small ones.
  - q rows are never gathered: within a dst block the q rows come from only
    128 nodes, so qi = S_T_tile @ Q_blk is a TensorE matmul with a
    host-precomputed one-hot S_T (S_T[d, e] = 1 iff edge e's dst is d).
  - Per-edge attention logits on Vector (reading qi straight from PSUM);
    exp on Scalar; the segment-softmax denominator and the weighted sum over
    incoming edges are ONE accumulated TensorE matmul with the one-hot
    scatter matrix S (S[e, d] = edge e's dst is block-node d). Padding edges
    have all-zero S rows so they drop out.
  - Small weights are replicated; k/v row tables are exchanged with an
    AllGather between the conv layers; the tiny Q-head is computed
    redundantly and combined with a masked AllReduce.
"""

import sys

sys.path.insert(0, "/opt/trn_rl_repo")

import numpy as np
import ml_dtypes

import concourse.bacc as bacc
import concourse.bass as bass
import concourse.mybir as mybir
import concourse.tile as tile
from concourse import bass_utils, library_config

F32 = mybir.dt.float32
BF16 = mybir.dt.bfloat16
I16 = mybir.dt.int16

N_CORES = 8


class Cfg:
    def __init__(self, n_nodes=50000, nblk=49, b=64, edge_bf16=True,
                 chunk_tiles=80, qi_group=6):
        self.N = n_nodes
        self.NBLK = nblk                 # dst blocks per core
        self.SHARD = nblk * 128          # nodes per core
        self.N_PAD = 8 * self.SHARD
        self.HALF = self.N_PAD // 2      # kv table split (int16 gather idx)
        self.B = b                       # batch (selected nodes)
        self.F_IN = 128
        self.H = 32
        self.HD = 128
        self.HEADS = 4
        self.EPS = 1e-16
        self.SCALE = 1.0 / np.sqrt(32.0)
        self.edge_bf16 = edge_bf16
        self.EDT = BF16 if edge_bf16 else F32
        self.EDT_NP = ml_dtypes.bfloat16 if edge_bf16 else np.float32
        self.CHUNK_TILES = chunk_tiles   # max 128-edge tiles per kv chunk
        self.QI_GROUP = qi_group         # tiles per qi PSUM group
        assert self.N <= self.N_PAD and self.HALF < 32768


# --------------------------------------------------------------------------
# host-side preprocessing
# --------------------------------------------------------------------------


def _wrap16(values, slots):
    """dma_gather idx layout: idx i lives at [i % 16, i // 16], replicated
    across the eight 16-partition groups."""
    arr = np.zeros((16, slots // 16), dtype=np.int16)
    arr[np.arange(len(values)) % 16, np.arange(len(values)) // 16] = values
    return np.tile(arr, (8, 1))


def _plan_chunks(cfg, t_lo, t_hi):
    """Greedily group consecutive blocks into chunks of <= CHUNK_TILES tiles.

    Returns chunks: list of dicts with
      blocks: list of block ids
      t_lo_sum / t_hi_sum: tiles in the chunk's lo / hi region
      tile0: global tile index where this chunk starts
      runs[b]: per block, two (region_tile_off_in_chunk, ntiles) runs
    and the global tile layout [chunk0: lo tiles (block major) | hi tiles
    (block major)] [chunk1: ...].
    """
    chunks = []
    b = 0
    tile0 = 0
    while b < cfg.NBLK:
        blocks = []
        tl = th = 0
        while b < cfg.NBLK:
            need = t_lo[b] + t_hi[b]
            if blocks and tl + th + need > cfg.CHUNK_TILES:
                break
            blocks.append(b)
            tl += t_lo[b]
            th += t_hi[b]
            b += 1
        runs = {}
        lo_off = 0
        hi_off = tl
        for blk in blocks:
            runs[blk] = ((lo_off, t_lo[blk]), (hi_off, t_hi[blk]))
            lo_off += t_lo[blk]
            hi_off += t_hi[blk]
        chunks.append(dict(blocks=blocks, t_lo_sum=tl, t_hi_sum=th,
                           tile0=tile0, runs=runs))
        tile0 += tl + th
    return chunks, tile0


def _prep_edges(cfg, edge_index):
    src = np.ascontiguousarray(edge_index[0]).astype(np.int64)
    dst = np.ascontiguousarray(edge_index[1]).astype(np.int64)
    core = dst // cfg.SHARD
    blk = (dst % cfg.SHARD) // 128
    hi = (src >= cfg.HALF).astype(np.int64)

    cnt = np.zeros((N_CORES, cfg.NBLK, 2), np.int64)
    np.add.at(cnt, (core, blk, hi), 1)
    t_lo = np.maximum(1, (cnt[:, :, 0].max(0) + 127) // 128)  # [NBLK]
    t_hi = np.maximum(1, (cnt[:, :, 1].max(0) + 127) // 128)

    chunks, total_tiles = _plan_chunks(cfg, t_lo.tolist(), t_hi.tolist())
    slots = total_tiles * 128
    assert slots % 16 == 0

    # global slot base for each (blk, hi) group
    grp_base = np.zeros((cfg.NBLK, 2), np.int64)
    for ch in chunks:
        for blk_id in ch["blocks"]:
            (lo_off, _), (hi_off, _) = ch["runs"][blk_id]
            grp_base[blk_id, 0] = (ch["tile0"] + lo_off) * 128
            grp_base[blk_id, 1] = (ch["tile0"] + hi_off) * 128

    chunk_of_blk = np.zeros(cfg.NBLK, np.int64)
    for ci, ch in enumerate(chunks):
        for blk_id in ch["blocks"]:
            chunk_of_blk[blk_id] = ci

    order = np.lexsort((src, blk, hi, chunk_of_blk[blk], core))
    s_src, s_dst, s_core, s_blk, s_hi = (
        src[order], dst[order], core[order], blk[order], hi[order])

    per_core = []
    for c in range(N_CORES):
        m = s_core == c
        csrc, cdst, cblk, chi = s_src[m], s_dst[m], s_blk[m], s_hi[m]
        # composite key non-decreasing under the sort above
        key = (chunk_of_blk[cblk] * 2 + chi) * 64 + cblk
        kcounts = np.bincount(key, minlength=(len(chunks) * 2) * 64)
        kstarts = np.zeros_like(kcounts)
        kstarts[1:] = np.cumsum(kcounts)[:-1]
        rank = np.arange(len(key)) - kstarts[key]
        slot = grp_base[cblk, chi] + rank

        kv_val = np.where(chi == 1, csrc - cfg.HALF, csrc)
        kv_idx = np.zeros(slots, np.int64)
        kv_idx[slot] = kv_val

        S = np.zeros((128, slots), cfg.EDT_NP)
        scol = (slot // 128) * 128 + (cdst % 128)
        S[slot % 128, scol] = 1.0
        ST = np.zeros((128, slots), cfg.EDT_NP)
        stcol = (slot // 128) * 128 + (slot % 128)
        ST[cdst % 128, stcol] = 1.0

        per_core.append(dict(kv_idx=_wrap16(kv_idx, slots), S=S, ST=ST))
    return per_core, t_lo.tolist(), t_hi.tolist(), chunks, slots


def _prep_inputs(cfg, inputs):
    x = np.asarray(inputs["x"], np.float32)
    idx = np.asarray(inputs["idx"]).astype(np.int64)
    f32 = lambda k: np.ascontiguousarray(np.asarray(inputs[k], np.float32))

    xp = np.zeros((cfg.N_PAD, cfg.F_IN), np.float32)
    xp[: cfg.N] = x

    per_core_e, t_lo, t_hi, chunks, slots = _prep_edges(cfg, inputs["edge_index"])

    wkv1b = np.ascontiguousarray(np.concatenate([
        np.concatenate([f32("c1_wk"), f32("c1_wv")], axis=1),
        np.concatenate([f32("c1_bk"), f32("c1_bv")])[None, :]], axis=0))
    wq1b = np.ascontiguousarray(
        np.concatenate([f32("c1_wq"), f32("c1_bq")[None, :]], axis=0))
    enc_w2b = np.ascontiguousarray(
        np.concatenate([f32("enc_w2"), f32("enc_b2")[None, :]], axis=0))
    wkv2 = np.ascontiguousarray(
        np.concatenate([f32("c2_wk"), f32("c2_wv")], axis=1))     # [128,256]
    bkv2 = np.ascontiguousarray(
        np.concatenate([f32("c2_bk"), f32("c2_bv")])[None, :])
    qw1 = f32("q_w1")                                              # [288,128]
    bpad = ((cfg.B + 127) // 128) * 128

    in_maps = []
    for c in range(N_CORES):
        shard = slice(c * cfg.SHARD, (c + 1) * cfg.SHARD)
        own = (idx // cfg.SHARD) == c
        idx_loc = np.where(own, idx - c * cfg.SHARD, 0)
        im = dict(
            xT=np.ascontiguousarray(xp[shard].T),          # [128, SHARD]
            enc_w1=f32("enc_w1"),
            enc_b1=f32("enc_b1").reshape(32, 1),
            enc_w2=f32("enc_w2"),
            enc_b2c=f32("enc_b2").reshape(32, 1),
            enc_w2b=enc_w2b,
            wq1b=wq1b, wkv1b=wkv1b,
            wq2=f32("c2_wq"), bq2=np.ascontiguousarray(f32("c2_bq")[None, :]),
            wkv2=wkv2, bkv2=bkv2,
            qw1a=np.ascontiguousarray(qw1[0:32]),
            qw1b=np.ascontiguousarray(qw1[32:160]),
            qw1c=np.ascontiguousarray(qw1[160:288]),
            qb1=f32("q_b1").reshape(128, 1),
            qw2=f32("q_w2"),
            qb2=f32("q_b2").reshape(1, 2),
            ones128=np.ones((1, 128), np.float32),
            id128=np.eye(128, dtype=np.float32),
            kv_idx=per_core_e[c]["kv_idx"],
            S_all=per_core_e[c]["S"],
            ST_all=per_core_e[c]["ST"],
            idx_x=_wrap16(idx_loc, bpad),
            own_mask=own.astype(np.float32).reshape(cfg.B, 1),
        )
        in_maps.append(im)
    return in_maps, t_lo, t_hi, chunks, slots


# --------------------------------------------------------------------------
# device program
# --------------------------------------------------------------------------


def build_program(cfg, t_lo, t_hi, chunks, slots):
    nc = bacc.Bacc("TRN2", target_bir_lowering=False, debug=False,
                   num_devices=N_CORES)
    EDT = cfg.EDT
    NB, SH = cfg.NBLK, cfg.SHARD
    RG = [list(range(N_CORES))]
    RELU = mybir.ActivationFunctionType.Relu
    COPY = mybir.ActivationFunctionType.Copy
    EXP = mybir.ActivationFunctionType.Exp

    def din(name, shape, dt=F32):
        return nc.dram_tensor(name, list(shape), dt, kind="ExternalInput").ap()

    xT = din("xT", [128, SH])
    enc_w1 = din("enc_w1", [128, 32]); enc_b1 = din("enc_b1", [32, 1])
    enc_w2 = din("enc_w2", [32, 32]); enc_b2c = din("enc_b2c", [32, 1])
    enc_w2b = din("enc_w2b", [33, 32])
    wq1b = din("wq1b", [33, 128]); wkv1b = din("wkv1b", [33, 256])
    wq2 = din("wq2", [128, 128]); bq2 = din("bq2", [1, 128])
    wkv2 = din("wkv2", [128, 256]); bkv2 = din("bkv2", [1, 256])
    qw1a = din("qw1a", [32, 128]); qw1b = din("qw1b", [128, 128])
    qw1c = din("qw1c", [128, 128]); qb1 = din("qb1", [128, 1])
    qw2 = din("qw2", [128, 2]); qb2 = din("qb2", [1, 2])
    ones128 = din("ones128", [1, 128]); id128 = din("id128", [128, 128])
    kv_idx_d = din("kv_idx", [128, slots // 16], I16)
    S_d = din("S_all", [128, slots], EDT)
    ST_d = din("ST_all", [128, slots], EDT)
    bpad = ((cfg.B + 127) // 128) * 128
    idx_x_d = din("idx_x", [128, bpad // 16], I16)
    own_mask_d = din("own_mask", [cfg.B, 1])
    out_d = nc.dram_tensor("out", [cfg.B, 2], F32, kind="ExternalOutput").ap()

    with tile.TileContext(nc) as tc:
        with (
            tc.tile_pool(name="const", bufs=1) as cpool,
            tc.tile_pool(name="kvch", bufs=3) as kvpool,
            tc.tile_pool(name="work", bufs=2) as wpool,
            tc.tile_pool(name="small", bufs=3) as spool,
            tc.tile_pool(name="psA", bufs=2, space="PSUM") as psA,
            tc.tile_pool(name="psB", bufs=2, space="PSUM") as psB,
            tc.tile_pool(name="psC", bufs=2, space="PSUM") as psC,
            tc.tile_pool(name="dram", bufs=1, space="DRAM") as dpool,
        ):
            nc.gpsimd.load_library(library_config.mlp)

            def ld(ap, shape, dt=F32, nm=None):
                t = cpool.tile(shape, dt, name=nm or ("ld_" + ap.tensor.name))
                nc.sync.dma_start(t[:], ap[:])
                return t

            w_enc1 = ld(enc_w1, [128, 32]); b_enc1 = ld(enc_b1, [32, 1])
            w_enc2 = ld(enc_w2, [32, 32]); b_enc2c = ld(enc_b2c, [32, 1])
            w_enc2b = ld(enc_w2b, [33, 32])
            w_q1b = ld(wq1b, [33, 128]); w_kv1b = ld(wkv1b, [33, 256])
            w_q2 = ld(wq2, [128, 128]); b_q2 = ld(bq2, [1, 128])
            w_kv2 = ld(wkv2, [128, 256]); b_kv2 = ld(bkv2, [1, 256])
            w_qha = ld(qw1a, [32, 128]); w_qhb = ld(qw1b, [128, 128])
            w_qhc = ld(qw1c, [128, 128])
            b_qh = ld(qb1, [128, 1]); w_qh2 = ld(qw2, [128, 2])
            b_qh2 = ld(qb2, [1, 2])
            ones_s = ld(ones128, [1, 128]); id_s = ld(id128, [128, 128])
            nidx16 = (cfg.B + 15) // 16
            idxx_s = cpool.tile([128, nidx16], I16, name="idxx_s")
            nc.sync.dma_start(idxx_s[:], idx_x_d[:, 0:nidx16])
            mask_s = ld(own_mask_d, [cfg.B, 1])
            kvidx_s = cpool.tile([128, slots // 16], I16, name="kvidx_s")
            nc.sync.dma_start(kvidx_s[:], kv_idx_d[:])

            q1_tab = dpool.tile([SH, 128], EDT, name="q1_tab")
            q2_tab = dpool.tile([SH, 128], EDT, name="q2_tab")
            kv1_sh = dpool.tile([SH, 256], EDT, name="kv1_sh")
            kv2_sh = dpool.tile([SH, 256], EDT, name="kv2_sh")
            kv1_full = dpool.tile([cfg.N_PAD, 256], EDT, name="kv1_full",
                                  addr_space="Shared")
            kv2_full = dpool.tile([cfg.N_PAD, 256], EDT, name="kv2_full",
                                  addr_space="Shared")
            h0_rows = dpool.tile([SH, 64], F32, name="h0_rows")
            h1_rows = dpool.tile([SH, 128], F32, name="h1_rows")
            h2_rows = dpool.tile([SH, 128], F32, name="h2_rows")
            ar_in = dpool.tile([cfg.B, 2], F32, name="ar_in")
            ar_out = dpool.tile([cfg.B, 2], F32, name="ar_out",
                                addr_space="Shared")

            # ===== encoder + conv1 tables, 4-block groups, fused biases =====
            def emit_tab_fused(hT1_blk, bsl, w_qb, w_kvb, q_tab, kv_sh):
                # hT1_blk: [c+1, 128] with trailing ones row; w_*b carry bias
                psq = psA.tile([128, 128], F32, tag="psA", name="ps_q")
                nc.tensor.matmul(psq[:], hT1_blk, w_qb[:], start=True,
                                 stop=True)
                qr = spool.tile([128, 128], EDT, tag="qr", name="qr")
                nc.scalar.activation(qr[:], psq[:], COPY)
                nc.sync.dma_start(q_tab[bsl, :], qr[:])

                psk = psB.tile([128, 256], F32, tag="psB", name="ps_kv")
                nc.tensor.matmul(psk[:], hT1_blk, w_kvb[:], start=True,
                                 stop=True)
                kvr = spool.tile([128, 256], EDT, tag="kvr", name="kvr")
                nc.vector.tensor_copy(kvr[:], psk[:])
                nc.sync.dma_start(kv_sh[bsl, :], kvr[:])

            ENC_G = 4
            for g0 in range(0, NB, ENC_G):
                nb = min(ENC_G, NB - g0)
                W = nb * 128
                gsl = slice(g0 * 128, g0 * 128 + W)
                xch = wpool.tile([128, ENC_G * 128], F32, tag="xch",
                                 name="xch")
                nc.sync.dma_start(xch[:, 0:W], xT[:, gsl])
                ps1 = psA.tile([32, ENC_G * 128], F32, tag="psA",
                               name="ps_enc1")
                nc.tensor.matmul(ps1[:, 0:W], w_enc1[:], xch[:, 0:W],
                                 start=True, stop=True)
                h1p = wpool.tile([33, ENC_G * 128], F32, tag="h1p",
                                 name="h1p")
                nc.vector.memset(h1p[32:33, :], 1.0)
                nc.scalar.activation(h1p[0:32, 0:W], ps1[:, 0:W], RELU,
                                     bias=b_enc1[:], scale=1.0)
                ps2 = psA.tile([32, ENC_G * 128], F32, tag="psA",
                               name="ps_enc2")
                nc.tensor.matmul(ps2[:, 0:W], w_enc2[:], h1p[0:32, 0:W],
                                 start=True, stop=True)
                h0b = wpool.tile([33, ENC_G * 128], F32, tag="h0b",
                                 name="h0b")
                nc.vector.memset(h0b[32:33, :], 1.0)
                nc.scalar.activation(h0b[0:32, 0:W], ps2[:, 0:W], RELU,
                                     bias=b_enc2c[:], scale=1.0)
                for j in range(nb):
                    b = g0 + j
                    bsl = slice(b * 128, (b + 1) * 128)
                    jsl = slice(j * 128, (j + 1) * 128)
                    # h0 rows (for the final x1 = h[idx] row gather)
                    psr = psC.tile([128, 32], F32, tag="psC", name="ps_h0r")
                    nc.tensor.matmul(psr[:], h1p[:, jsl], w_enc2b[:],
                                     start=True, stop=True)
                    h0r = spool.tile([128, 64], F32, tag="h0r", name="h0r")
                    nc.scalar.activation(h0r[:, 0:32], psr[:], RELU)
                    nc.sync.dma_start(h0_rows[bsl, :], h0r[:])
                    # conv1 q/kv table rows
                    emit_tab_fused(h0b[:, jsl], bsl, w_q1b, w_kv1b,
                                   q1_tab, kv1_sh)
            nc.gpsimd.collective_compute(
                "AllGather", mybir.AluOpType.bypass, replica_groups=RG,
                ins=[kv1_sh.opt()], outs=[kv1_full.opt()])

            # ================= conv layers =================
            def emit_conv(q_tab, kv_full, h_rows_out, emit_tab2):
                for ch in chunks:
                    TL, TH = ch["t_lo_sum"], ch["t_hi_sum"]
                    TC = TL + TH
                    tile0 = ch["tile0"]
                    oc = tile0 * 8  # idx cols (16 idx per col)

                    kv_ch = kvpool.tile([128, TC, 256], EDT, tag="kv_ch",
                                        name="kv_ch")
                    nc.gpsimd.dma_gather(
                        kv_ch[:, 0:TL, :], kv_full[0: cfg.HALF, :],
                        kvidx_s[:, oc: oc + TL * 8], TL * 128, TL * 128, 256,
                        single_packet=False)
                    nc.gpsimd.dma_gather(
                        kv_ch[:, TL:TC, :], kv_full[cfg.HALF: cfg.N_PAD, :],
                        kvidx_s[:, oc + TL * 8: oc + TC * 8], TH * 128,
                        TH * 128, 256, single_packet=False)

                    for b in ch["blocks"]:
                        runs = [r for r in ch["runs"][b] if r[1] > 0]
                        T = sum(r[1] for r in runs)
                        # block's S / S_T tiles (per run, from global layout)
                        s_tiles = []
                        st_tiles = []
                        for off, nt in runs:
                            g0 = (tile0 + off) * 128
                            s_t = wpool.tile([128, nt * 128], EDT, tag="S_b",
                                             name="S_b")
                            nc.sync.dma_start(s_t[:], S_d[:, g0: g0 + nt * 128])
                            s_tiles.append(s_t)
                            st_t = wpool.tile([128, nt * 128], EDT,
                                              tag="ST_b", name="ST_b")
                            nc.sync.dma_start(st_t[:],
                                              ST_d[:, g0: g0 + nt * 128])
                            st_tiles.append(st_t)
                        bsl = slice(b * 128, (b + 1) * 128)
                        q_blk = wpool.tile([128, 128], EDT, tag="q_blk",
                                           name="q_blk")
                        nc.sync.dma_start(q_blk[:], q_tab[bsl, :])

                        l_t = spool.tile([128, T * 4], F32, tag="l_t",
                                         name="l_t")
                        # qi via S_T @ Q_blk, prod+reduce per <=QI_GROUP tiles
                        lpos = 0
                        for (off, nt), st_t in zip(runs, st_tiles):
                            for g0 in range(0, nt, cfg.QI_GROUP):
                                g = min(cfg.QI_GROUP, nt - g0)
                                qi_ps = psA.tile([128, g, 128], F32,
                                                 tag="psA", name="qi_ps")
                                for i in range(g):
                                    c0 = (g0 + i) * 128
                                    nc.tensor.matmul(
                                        qi_ps[:, i, :],
                                        st_t[:, c0: c0 + 128], q_blk[:],
                                        start=True, stop=True)
                                prod = spool.tile([128, g, 128], EDT,
                                                  tag="prod", name="prod")
                                nc.vector.tensor_tensor(
                                    prod[:],
                                    qi_ps[:],
                                    kv_ch[:, off + g0: off + g0 + g, 0:128],
                                    mybir.AluOpType.mult)
                                nc.vector.tensor_reduce(
                                    l_t[:, lpos: lpos + g * 4].rearrange(
                                        "p (t h) -> p t h", h=4),
                                    prod[:].rearrange(
                                        "p t (h j) -> p t h j", h=4, j=32),
                                    mybir.AxisListType.X, mybir.AluOpType.add)
                                lpos += g * 4

                        rhs = wpool.tile([128, T, 132], EDT, tag="rhs",
                                         name="rhs")
                        nc.scalar.activation(
                            rhs[:, :, 0:4],
                            l_t[:].rearrange("p (t h) -> p t h", h=4),
                            EXP, scale=float(cfg.SCALE))
                        aw = wpool.tile([128, T, 128], EDT, tag="aw",
                                        name="aw")
                        nc.scalar.activation(
                            aw[:].rearrange("p t (h j) -> p t h j", h=4, j=32),
                            l_t[:].rearrange("p (t h) -> p t h", h=4)
                                .unsqueeze(-1).broadcast_to([128, T, 4, 32]),
                            EXP, scale=float(cfg.SCALE))
                        tpos = 0
                        for off, nt in runs:
                            nc.vector.tensor_tensor(
                                rhs[:, tpos: tpos + nt, 4:132],
                                kv_ch[:, off: off + nt, 128:256],
                                aw[:, tpos: tpos + nt, :],
                                mybir.AluOpType.mult)
                            tpos += nt

                        sc_ps = psB.tile([128, 132], F32, tag="psB",
                                         name="sc_ps")
                        t_i = 0
                        for s_t, (off, nt) in zip(s_tiles, runs):
                            for i in range(nt):
                                nc.tensor.matmul(
                                    sc_ps[:], s_t[:, i * 128: (i + 1) * 128],
                                    rhs[:, t_i, :], start=(t_i == 0),
                                    stop=(t_i == T - 1))
                                t_i += 1

                        den = spool.tile([128, 4], F32, tag="den", name="den")
                        nc.vector.tensor_scalar_add(den[:], sc_ps[:, 0:4],
                                                    float(cfg.EPS))
                        rec = spool.tile([128, 4], F32, tag="rec", name="rec")
                        nc.vector.reciprocal(rec[:], den[:])
                        h_blk = spool.tile([128, 128], F32, tag="h_blk",
                                           name="h_blk")
                        for h in range(4):
                            nc.scalar.activation(
                                h_blk[:, h * 32: (h + 1) * 32],
                                sc_ps[:, 4 + h * 32: 4 + (h + 1) * 32],
                                RELU, scale=rec[:, h: h + 1])
                        nc.sync.dma_start(h_rows_out[bsl, :], h_blk[:])
                        if emit_tab2:
                            tr_ps = psC.tile([128, 128], F32, tag="psC",
                                             name="tr_ps")
                            nc.tensor.transpose(tr_ps[:], h_blk[:], id_s[:])
                            h1tb = spool.tile([128, 128], F32, tag="h1tb",
                                              name="h1tb")
                            nc.scalar.activation(h1tb[:], tr_ps[:], COPY)
                            # conv2 q/kv table rows, inline during conv1
                            psq = psA.tile([128, 128], F32, tag="psA",
                                           name="ps_q2")
                            nc.tensor.matmul(psq[:], h1tb[:], w_q2[:],
                                             start=True, stop=False)
                            nc.tensor.matmul(psq[:], ones_s[:], b_q2[:],
                                             start=False, stop=True)
                            qr = spool.tile([128, 128], EDT, tag="qr",
                                            name="qr2")
                            nc.scalar.activation(qr[:], psq[:], COPY)
                            nc.sync.dma_start(q2_tab[bsl, :], qr[:])
                            psk = psB.tile([128, 256], F32, tag="psB",
                                           name="ps_kv2")
                            nc.tensor.matmul(psk[:], h1tb[:], w_kv2[:],
                                             start=True, stop=False)
                            nc.tensor.matmul(psk[:], ones_s[:], b_kv2[:],
                                             start=False, stop=True)
                            kvr = spool.tile([128, 256], EDT, tag="kvr",
                                             name="kvr2")
                            nc.vector.tensor_copy(kvr[:], psk[:])
                            nc.sync.dma_start(kv2_sh[bsl, :], kvr[:])

            emit_conv(q1_tab, kv1_full, h1_rows, True)
            nc.gpsimd.collective_compute(
                "AllGather", mybir.AluOpType.bypass, replica_groups=RG,
                ins=[kv2_sh.opt()], outs=[kv2_full.opt()])

            emit_conv(q2_tab, kv2_full, h2_rows, False)

            # ================= Q head =================
            def gather_xT(tab, width):
                g = spool.tile([128, 1, width], F32, tag="gx", name="gx")
                nc.gpsimd.dma_gather(g[:], tab[:, :], idxx_s[:],
                                     cfg.B, cfg.B, width)
                tp = psC.tile([128, 128], F32, tag="psC", name="tp_x")
                nc.tensor.transpose(tp[0:width, 0: cfg.B], g[0: cfg.B, 0, :],
                                    id_s[0: cfg.B, 0: cfg.B])
                xt = spool.tile([128, cfg.B], F32, tag="xt", name="xt")
                nc.scalar.activation(xt[0:width, :], tp[0:width, 0: cfg.B],
                                     COPY)
                return xt

            x1t = gather_xT(h0_rows, 64)
            x2t = gather_xT(h1_rows, 128)
            x3t = gather_xT(h2_rows, 128)

            zh_ps = psA.tile([128, cfg.B], F32, tag="psA", name="zh_ps")
            nc.tensor.matmul(zh_ps[:], w_qha[:], x1t[0:32, :],
                             start=True, stop=False)
            nc.tensor.matmul(zh_ps[:], w_qhb[:], x2t[0:128, :],
                             start=False, stop=False)
            nc.tensor.matmul(zh_ps[:], w_qhc[:], x3t[0:128, :],
                             start=False, stop=True)
            zh = spool.tile([128, cfg.B], F32, tag="zh", name="zh")
            nc.scalar.activation(zh[:], zh_ps[:], RELU, bias=b_qh[:],
                                 scale=1.0)
            o_ps = psB.tile([cfg.B, 2], F32, tag="psB", name="o_ps")
            nc.tensor.matmul(o_ps[:], zh[:], w_qh2[:], start=True, stop=False)
            nc.tensor.matmul(o_ps[:], ones_s[:, 0: cfg.B], b_qh2[:],
                             start=False, stop=True)
            ob = spool.tile([cfg.B, 2], F32, tag="ob", name="ob")
            nc.vector.tensor_scalar_mul(ob[:], o_ps[:], mask_s[:])
            nc.sync.dma_start(ar_in[:, :], ob[:])
            nc.gpsimd.collective_compute(
                "AllReduce", mybir.AluOpType.add, replica_groups=RG,
                ins=[ar_in.opt()], outs=[ar_out.opt()])
            nc.sync.dma_start(out_d[:, :], ar_out[:, :])

    nc.compile()
    return nc


# --------------------------------------------------------------------------
# entry point
# --------------------------------------------------------------------------

_trace_flag = {"trace": False}
_last = {}


def _chunk_key(chunks):
    return tuple((tuple(ch["blocks"]), ch["t_lo_sum"], ch["t_hi_sum"])
                 for ch in chunks)


def _run(inputs, cfg=None):
    cfg = cfg or Cfg()
    in_maps, t_lo, t_hi, chunks, slots = _prep_inputs(cfg, inputs)
    key = (slots, tuple(t_lo), tuple(t_hi), _chunk_key(chunks), cfg.edge_bf16)
    if _last.get("key") != key:
        _last["nc"] = build_program(cfg, t_lo, t_hi, chunks, slots)
        _last["key"] = key
    nc = _last["nc"]
    res = bass_utils.run_bass_kernel_spmd(
        nc, in_maps, core_ids=list(range(N_CORES)),
        trace=_trace_flag["trace"])
    _last["res"] = res
    return res.results[0]["out"].astype(np.float32)


def kernel(**inputs):
    return _run(inputs)
